# revision 2
# baseline (speedup 1.0000x reference)
"""Distributed multi-head attention kernel for one TRN2 chip (8 NeuronCores).

Problem: B=2, S=2048, D=1024, H=16 heads (dh=64), interleaved head split
(reshape d -> (dh, H) with heads LAST), scale = 1/sqrt(D).

Sharding: core c => batch b = c//4, head-group hg = c%4 (4 heads each).
No collectives: every core computes its own [s, 256] output slice.

Host-side marshalling:
  - weight columns permuted so each head's 64 columns are contiguous
  - x[b] pre-transposed to xT [D, S] (PE contracts over partitions, so x
    must be d-major; transposing on host is free)
  - bf16 casts for all matmul operands (fp32 PE matmul is multi-pass slow)

Device-side (per core, SPMD):
  - QT[dq, s] = Wq.T @ x.T (+bias), KT likewise, V[s, dv] with an extra
    ones column per head (V_aug) so PV also produces softmax row sums
  - per head: ST[j, i] = KT_h.T @ QT_h (K=64); E = exp(ST/32) on ScalarE
    straight out of PSUM (scores are tiny, |s| < ~0.3, so softmax without
    max subtraction is numerically safe)
  - OT_aug[65, i] += V_aug[j,:].T @ E[j, i] accumulated over j tiles;
    row 64 = sum_j E (softmax denominator)
  - normalize: DVE reciprocal of row 64, GPSIMD partition_broadcast,
    DVE multiply; DMA out OT [64(c), 2048(s)] per head (host transposes)
"""

import sys
import os

for _p in ("/opt/trn_rl_repo",):
    if os.path.isdir(_p) and _p not in sys.path:
        sys.path.insert(0, _p)

import numpy as np
import ml_dtypes
from contextlib import ExitStack

import concourse.bass as bass
import concourse.mybir as mybir
import concourse.tile as tile
from concourse import bacc
from concourse.bass_utils import run_bass_kernel_spmd

BF16 = mybir.dt.bfloat16
F32 = mybir.dt.float32
NPBF16 = ml_dtypes.bfloat16

B, S, D, H = 2, 2048, 1024, 16
NCORES = 8
HGROUPS = 4              # tensor-parallel ways over heads
NH_LOC = H // HGROUPS    # 4 heads per core
DH = D // H              # 64
DQ = NH_LOC * DH         # 256 projection cols per core
KT = D // 128            # 8 contraction tiles
SCALE = 1.0 / 32.0       # 1/sqrt(D)

# column permutation: permuted col h*64+c  <-  original col c*16+h
PERM = np.array([c * H + h for h in range(H) for c in range(DH)], dtype=np.int64)


def build_bass():
    nc = bacc.Bacc("TRN2", target_bir_lowering=False)
    xT_d = nc.dram_tensor("xT", [D, S], BF16, kind="ExternalInput")
    wq_d = nc.dram_tensor("wq", [D, DQ], BF16, kind="ExternalInput")
    wk_d = nc.dram_tensor("wk", [D, DQ], BF16, kind="ExternalInput")
    wv_d = nc.dram_tensor("wv", [D, DQ], BF16, kind="ExternalInput")
    bqT_d = nc.dram_tensor("bqT", [DQ, 1], F32, kind="ExternalInput")
    out_d = nc.dram_tensor("out", [DQ, S], F32, kind="ExternalOutput")

    with ExitStack() as ctx:
        tc = ctx.enter_context(tile.TileContext(nc))
        consts = ctx.enter_context(tc.tile_pool(name="consts", bufs=1))
        xpool = ctx.enter_context(tc.tile_pool(name="xpool", bufs=KT))
        proj = ctx.enter_context(tc.tile_pool(name="proj", bufs=1))
        epool = ctx.enter_context(tc.tile_pool(name="epool", bufs=3))
        npool = ctx.enter_context(tc.tile_pool(name="npool", bufs=2))
        opool = ctx.enter_context(tc.tile_pool(name="opool", bufs=2))

        # ---- load inputs ----
        xT_sb = [xpool.tile([128, S], BF16, tag="xT", name=f"xT{_i}") for _i in range(KT)]
        for kt in range(KT):
            nc.sync.dma_start(out=xT_sb[kt][:], in_=xT_d[kt * 128:(kt + 1) * 128, :])

        wq_sb = consts.tile([128, KT, DQ], BF16)
        wk_sb = consts.tile([128, KT, DQ], BF16)
        wv_sb = consts.tile([128, KT, DQ], BF16)
        nc.sync.dma_start(out=wq_sb[:], in_=wq_d.ap().rearrange("(t p) n -> p t n", p=128))
        nc.sync.dma_start(out=wk_sb[:], in_=wk_d.ap().rearrange("(t p) n -> p t n", p=128))
        nc.sync.dma_start(out=wv_sb[:], in_=wv_d.ap().rearrange("(t p) n -> p t n", p=128))
        bq_sb = consts.tile([128, 2, 1], F32)
        nc.sync.dma_start(out=bq_sb[:], in_=bqT_d.ap().rearrange("(t p) o -> p t o", p=128))

        # QT/KT: [128(dq within tile), 2 tiles, 2048(s)]; V_aug: per s-tile
        # [4 heads x 65] with col 64 of each head == 1.0 (memset then 64-wide copies)
        qt_sb = consts.tile([128, 2, S], BF16)
        kt_sb = consts.tile([128, 2, S], BF16)
        v_sb = consts.tile([128, 16, NH_LOC * (DH + 1)], BF16)
        nc.vector.memset(v_sb[:], 1.0)

        # ---- projections ----
        with tc.tile_pool(name="pproj", bufs=4, space="PSUM") as pproj:
            for m in range(2):          # dq tile (128 cols each)
                for ic in range(4):     # s chunk of 512
                    ps = pproj.tile([128, 512], F32, tag="pp")
                    for kt in range(KT):
                        nc.tensor.matmul(
                            ps[:],
                            lhsT=wq_sb[:, kt, m * 128:(m + 1) * 128],
                            rhs=xT_sb[kt][:, ic * 512:(ic + 1) * 512],
                            start=(kt == 0), stop=(kt == KT - 1),
                        )
                    nc.vector.tensor_scalar_add(
                        qt_sb[:, m, ic * 512:(ic + 1) * 512], ps[:], bq_sb[:, m, :])
            for m in range(2):
                for ic in range(4):
                    ps = pproj.tile([128, 512], F32, tag="pp")
                    for kt in range(KT):
                        nc.tensor.matmul(
                            ps[:],
                            lhsT=wk_sb[:, kt, m * 128:(m + 1) * 128],
                            rhs=xT_sb[kt][:, ic * 512:(ic + 1) * 512],
                            start=(kt == 0), stop=(kt == KT - 1),
                        )
                    nc.vector.tensor_copy(
                        out=kt_sb[:, m, ic * 512:(ic + 1) * 512], in_=ps[:])
            for st in range(16):        # s tile of 128
                ps = pproj.tile([128, 512], F32, tag="pp")
                for kt in range(KT):
                    nc.tensor.matmul(
                        ps[:, 0:DQ],
                        lhsT=xT_sb[kt][:, st * 128:(st + 1) * 128],
                        rhs=wv_sb[:, kt, :],
                        start=(kt == 0), stop=(kt == KT - 1),
                    )
                nc.vector.tensor_copy(
                    out=v_sb[:, st, :].rearrange("p (h e) -> p h e", e=DH + 1)[:, :, 0:DH],
                    in_=ps[:, 0:DQ].rearrange("p (h c) -> p h c", c=DH),
                )

        # ---- attention, head by head ----
        with (
            tc.tile_pool(name="psc", bufs=1, space="PSUM") as psc,
            tc.tile_pool(name="pov", bufs=1, space="PSUM") as pov,
        ):
            for h in range(NH_LOC):
                m = h // 2
                off = (h % 2) * DH
                o_ps = pov.tile([DH + 1, S], F32, tag="ov")
                for jc in range(16):
                    s_ps = psc.tile([128, S], F32, tag="sc")
                    for ic in range(4):
                        nc.tensor.matmul(
                            s_ps[:, ic * 512:(ic + 1) * 512],
                            lhsT=kt_sb[off:off + DH, m, jc * 128:(jc + 1) * 128],
                            rhs=qt_sb[off:off + DH, m, ic * 512:(ic + 1) * 512],
                            start=True, stop=True,
                        )
                    e_sb = epool.tile([128, S], BF16, tag="e")
                    nc.scalar.activation(
                        e_sb[:], s_ps[:], mybir.ActivationFunctionType.Exp, scale=SCALE)
                    for ic in range(4):
                        nc.tensor.matmul(
                            o_ps[:, ic * 512:(ic + 1) * 512],
                            lhsT=v_sb[:, jc, h * (DH + 1):(h + 1) * (DH + 1)],
                            rhs=e_sb[:, ic * 512:(ic + 1) * 512],
                            start=(jc == 0), stop=(jc == 15),
                        )
                # normalize: recip(rowsum) -> broadcast across partitions -> mul
                rl_sb = npool.tile([1, S], F32, tag="rl")
                nc.vector.reciprocal(rl_sb[:], o_ps[DH:DH + 1, :])
                rb_sb = npool.tile([DH, S], F32, tag="rb")
                nc.gpsimd.partition_broadcast(rb_sb[:], rl_sb[:])
                ost = opool.tile([DH, S], F32, tag="ost")
                nc.vector.tensor_mul(ost[:], o_ps[0:DH, :], rb_sb[:])
                nc.sync.dma_start(out=out_d[h * DH:(h + 1) * DH, :], in_=ost[:])

    nc.finalize()
    return nc


_NC_CACHE = None


def _get_nc():
    global _NC_CACHE
    if _NC_CACHE is None:
        _NC_CACHE = build_bass()
    return _NC_CACHE


def kernel(x, Wq, Bq, Wk, Wv, n_heads=16, **_ignored):
    x = np.asarray(x, dtype=np.float32)
    Wq = np.asarray(Wq, dtype=np.float32)
    Bq = np.asarray(Bq, dtype=np.float32).reshape(-1)
    Wk = np.asarray(Wk, dtype=np.float32)
    Wv = np.asarray(Wv, dtype=np.float32)

    wq_p = Wq[:, PERM]
    wk_p = Wk[:, PERM]
    wv_p = Wv[:, PERM]
    bq_p = Bq[PERM]

    xT = [np.ascontiguousarray(x[b].T).astype(NPBF16) for b in range(B)]
    in_maps = []
    for core in range(NCORES):
        b, hg = core // HGROUPS, core % HGROUPS
        sl = slice(hg * DQ, (hg + 1) * DQ)
        in_maps.append({
            "xT": xT[b],
            "wq": np.ascontiguousarray(wq_p[:, sl]).astype(NPBF16),
            "wk": np.ascontiguousarray(wk_p[:, sl]).astype(NPBF16),
            "wv": np.ascontiguousarray(wv_p[:, sl]).astype(NPBF16),
            "bqT": np.ascontiguousarray(bq_p[sl]).reshape(DQ, 1).astype(np.float32),
        })

    nc = _get_nc()
    res = run_bass_kernel_spmd(nc, in_maps, core_ids=list(range(NCORES)))

    out = np.empty((B, S, D), dtype=np.float32)
    for b in range(B):
        big = np.concatenate(
            [res.results[b * HGROUPS + hg]["out"] for hg in range(HGROUPS)], axis=0)
        out[b][:, PERM] = big.T
    return out


# revision 3
# speedup vs baseline: 1.5067x; 1.5067x over previous
"""Distributed multi-head attention kernel for one TRN2 chip (8 NeuronCores).

Problem: B=2, S=2048, D=1024, H=16 heads (dh=64), interleaved head split
(reshape d -> (dh, H) with heads LAST), scale = 1/sqrt(D).

Sharding: core c => batch b = c//4, head-group hg = c%4 (4 heads each).
No collectives: every core computes its own [s, 256] output slice.

Host-side marshalling:
  - weight columns permuted so each head's 64 columns are contiguous
  - x[b] pre-transposed to xT [D, S] (PE contracts over partitions, so x
    must be d-major; transposing on host is free)
  - bf16 casts for all matmul operands (fp32 PE matmul is multi-pass slow)

Device-side (per core, SPMD):
  - QT[dq, s] = Wq.T @ x.T (+bias), KT likewise, V[s, dv] with an extra
    ones column per head (V_aug) so PV also produces softmax row sums
  - per head: ST[j, i] = KT_h.T @ QT_h (K=64); E = exp(ST/32) on ScalarE
    straight out of PSUM (scores are tiny, |s| < ~0.3, so softmax without
    max subtraction is numerically safe)
  - OT_aug[65, i] += V_aug[j,:].T @ E[j, i] accumulated over j tiles;
    row 64 = sum_j E (softmax denominator)
  - normalize: DVE reciprocal of row 64, GPSIMD partition_broadcast,
    DVE multiply; DMA out OT [64(c), 2048(s)] per head (host transposes)
"""

import sys
import os

for _p in ("/opt/trn_rl_repo",):
    if os.path.isdir(_p) and _p not in sys.path:
        sys.path.insert(0, _p)

import numpy as np
import ml_dtypes
from contextlib import ExitStack

import concourse.bass as bass
import concourse.mybir as mybir
import concourse.tile as tile
from concourse import bacc
from concourse.bass_utils import run_bass_kernel_spmd

BF16 = mybir.dt.bfloat16
F32 = mybir.dt.float32
NPBF16 = ml_dtypes.bfloat16

B, S, D, H = 2, 2048, 1024, 16
NCORES = 8
HGROUPS = 4              # tensor-parallel ways over heads
NH_LOC = H // HGROUPS    # 4 heads per core
DH = D // H              # 64
DQ = NH_LOC * DH         # 256 projection cols per core
KT = D // 128            # 8 contraction tiles
SCALE = 1.0 / 32.0       # 1/sqrt(D)

# column permutation: permuted col h*64+c  <-  original col c*16+h
PERM = np.array([c * H + h for h in range(H) for c in range(DH)], dtype=np.int64)


def build_bass():
    nc = bacc.Bacc("TRN2", target_bir_lowering=False)
    xT_d = nc.dram_tensor("xT", [D, S], BF16, kind="ExternalInput")
    wq_d = nc.dram_tensor("wq", [D, DQ], BF16, kind="ExternalInput")
    wk_d = nc.dram_tensor("wk", [D, DQ], BF16, kind="ExternalInput")
    wv_d = nc.dram_tensor("wv", [D, DQ], BF16, kind="ExternalInput")
    bqT_d = nc.dram_tensor("bqT", [DQ, 1], F32, kind="ExternalInput")
    out_d = nc.dram_tensor("out", [DQ, S], F32, kind="ExternalOutput")

    with ExitStack() as ctx:
        tc = ctx.enter_context(tile.TileContext(nc))
        consts = ctx.enter_context(tc.tile_pool(name="consts", bufs=1))
        xpool = ctx.enter_context(tc.tile_pool(name="xpool", bufs=KT))
        epool = ctx.enter_context(tc.tile_pool(name="epool", bufs=6))
        npool = ctx.enter_context(tc.tile_pool(name="npool", bufs=2))
        opool = ctx.enter_context(tc.tile_pool(name="opool", bufs=2))
        # one flat PSUM layout, no nested pools (nested release would
        # serialize attention behind all projection work):
        # pmain: 2 bufs x [128,1024] (2 banks each) shared by projection
        # accumulators and score tiles; pov: [65,2048] PV accumulator (4 banks)
        pmain = ctx.enter_context(tc.tile_pool(name="pmain", bufs=2, space="PSUM"))
        pov = ctx.enter_context(tc.tile_pool(name="pov", bufs=1, space="PSUM"))

        # ---- load inputs ----
        xT_sb = [xpool.tile([128, S], BF16, tag="xT", name=f"xT{_i}") for _i in range(KT)]
        for kt in range(KT):
            nc.sync.dma_start(out=xT_sb[kt][:], in_=xT_d[kt * 128:(kt + 1) * 128, :])

        wq_sb = consts.tile([128, KT, DQ], BF16)
        wk_sb = consts.tile([128, KT, DQ], BF16)
        wv_sb = consts.tile([128, KT, DQ], BF16)
        nc.sync.dma_start(out=wq_sb[:], in_=wq_d.ap().rearrange("(t p) n -> p t n", p=128))
        nc.sync.dma_start(out=wk_sb[:], in_=wk_d.ap().rearrange("(t p) n -> p t n", p=128))
        nc.sync.dma_start(out=wv_sb[:], in_=wv_d.ap().rearrange("(t p) n -> p t n", p=128))
        bq_sb = consts.tile([128, 2, 1], F32)
        nc.sync.dma_start(out=bq_sb[:], in_=bqT_d.ap().rearrange("(t p) o -> p t o", p=128))

        qt_sb = consts.tile([128, 2, S], BF16)
        kt_sb = consts.tile([128, 2, S], BF16)
        v_sb = consts.tile([128, 16, NH_LOC * (DH + 1)], BF16)
        nc.vector.memset(v_sb[:], 1.0)

        def proj_qk(m, ic):
            """project s-chunk ic (512 wide) of QT and KT tile m"""
            ps = pmain.tile([128, 512], F32, tag="pm", name="psq")
            for kt in range(KT):
                nc.tensor.matmul(
                    ps[:],
                    lhsT=wq_sb[:, kt, m * 128:(m + 1) * 128],
                    rhs=xT_sb[kt][:, ic * 512:(ic + 1) * 512],
                    start=(kt == 0), stop=(kt == KT - 1),
                )
            nc.vector.tensor_scalar_add(
                qt_sb[:, m, ic * 512:(ic + 1) * 512], ps[:], bq_sb[:, m, :])
            ps = pmain.tile([128, 512], F32, tag="pm", name="psk")
            for kt in range(KT):
                nc.tensor.matmul(
                    ps[:],
                    lhsT=wk_sb[:, kt, m * 128:(m + 1) * 128],
                    rhs=xT_sb[kt][:, ic * 512:(ic + 1) * 512],
                    start=(kt == 0), stop=(kt == KT - 1),
                )
            nc.vector.tensor_copy(out=kt_sb[:, m, ic * 512:(ic + 1) * 512], in_=ps[:])

        def proj_v(st):
            """project V for s-tile st into v_sb (leaving the ones columns)"""
            ps = pmain.tile([128, 512], F32, tag="pm", name="psv")
            for kt in range(KT):
                nc.tensor.matmul(
                    ps[:, 0:DQ],
                    lhsT=xT_sb[kt][:, st * 128:(st + 1) * 128],
                    rhs=wv_sb[:, kt, :],
                    start=(kt == 0), stop=(kt == KT - 1),
                )
            nc.vector.tensor_copy(
                out=v_sb[:, st, :].rearrange("p (h e) -> p h e", e=DH + 1)[:, :, 0:DH],
                in_=ps[:, 0:DQ].rearrange("p (h c) -> p h c", c=DH),
            )

        # QT/KT tile m=0 up front (heads 0/1 depend on it) ...
        for ic in range(4):
            proj_qk(0, ic)

        # ... then heads, with the remaining projection work interleaved into
        # the first two head loops so PE stays dense while ACT runs exp.
        for h in range(NH_LOC):
            m = h // 2
            off = (h % 2) * DH
            o_ps = pov.tile([DH + 1, S], F32, tag="ov", name="ops")
            for jc in range(16):
                if h == 0:
                    proj_v(jc)                 # PV(h=0, jc) needs exactly this
                elif h == 1 and jc < 8:
                    proj_qk(1, jc // 2)        # heads 2/3 inputs
                for half in range(2):
                    s_ps = pmain.tile([128, 1024], F32, tag="pm", name="pss")
                    for ic2 in range(2):
                        i0 = half * 1024 + ic2 * 512
                        nc.tensor.matmul(
                            s_ps[:, ic2 * 512:(ic2 + 1) * 512],
                            lhsT=kt_sb[off:off + DH, m, jc * 128:(jc + 1) * 128],
                            rhs=qt_sb[off:off + DH, m, i0:i0 + 512],
                            start=True, stop=True,
                        )
                    e_sb = epool.tile([128, 1024], BF16, tag="e", name="esb")
                    nc.scalar.activation(
                        e_sb[:], s_ps[:], mybir.ActivationFunctionType.Exp, scale=SCALE)
                    for ic2 in range(2):
                        i0 = half * 1024 + ic2 * 512
                        nc.tensor.matmul(
                            o_ps[:, i0:i0 + 512],
                            lhsT=v_sb[:, jc, h * (DH + 1):(h + 1) * (DH + 1)],
                            rhs=e_sb[:, ic2 * 512:(ic2 + 1) * 512],
                            start=(jc == 0), stop=(jc == 15),
                        )
            # normalize: recip(rowsum) -> broadcast across partitions -> mul
            rl_sb = npool.tile([1, S], F32, tag="rl")
            nc.vector.reciprocal(rl_sb[:], o_ps[DH:DH + 1, :])
            rb_sb = npool.tile([DH, S], F32, tag="rb")
            nc.gpsimd.partition_broadcast(rb_sb[:], rl_sb[:])
            ost = opool.tile([DH, S], F32, tag="ost")
            nc.vector.tensor_mul(ost[:], o_ps[0:DH, :], rb_sb[:])
            nc.sync.dma_start(out=out_d[h * DH:(h + 1) * DH, :], in_=ost[:])

    nc.finalize()
    return nc


_NC_CACHE = None


def _get_nc():
    global _NC_CACHE
    if _NC_CACHE is None:
        _NC_CACHE = build_bass()
    return _NC_CACHE


def kernel(x, Wq, Bq, Wk, Wv, n_heads=16, **_ignored):
    x = np.asarray(x, dtype=np.float32)
    Wq = np.asarray(Wq, dtype=np.float32)
    Bq = np.asarray(Bq, dtype=np.float32).reshape(-1)
    Wk = np.asarray(Wk, dtype=np.float32)
    Wv = np.asarray(Wv, dtype=np.float32)

    wq_p = Wq[:, PERM]
    wk_p = Wk[:, PERM]
    wv_p = Wv[:, PERM]
    bq_p = Bq[PERM]

    xT = [np.ascontiguousarray(x[b].T).astype(NPBF16) for b in range(B)]
    in_maps = []
    for core in range(NCORES):
        b, hg = core // HGROUPS, core % HGROUPS
        sl = slice(hg * DQ, (hg + 1) * DQ)
        in_maps.append({
            "xT": xT[b],
            "wq": np.ascontiguousarray(wq_p[:, sl]).astype(NPBF16),
            "wk": np.ascontiguousarray(wk_p[:, sl]).astype(NPBF16),
            "wv": np.ascontiguousarray(wv_p[:, sl]).astype(NPBF16),
            "bqT": np.ascontiguousarray(bq_p[sl]).reshape(DQ, 1).astype(np.float32),
        })

    nc = _get_nc()
    res = run_bass_kernel_spmd(nc, in_maps, core_ids=list(range(NCORES)))

    out = np.empty((B, S, D), dtype=np.float32)
    for b in range(B):
        big = np.concatenate(
            [res.results[b * HGROUPS + hg]["out"] for hg in range(HGROUPS)], axis=0)
        out[b][:, PERM] = big.T
    return out


# revision 4
# speedup vs baseline: 1.7047x; 1.1314x over previous
"""Distributed multi-head attention kernel for one TRN2 chip (8 NeuronCores).

Problem: B=2, S=2048, D=1024, H=16 heads (dh=64), interleaved head split
(reshape d -> (dh, H) with heads LAST), scale = 1/sqrt(D).

Sharding: core c => batch b = c//4, head-group hg = c%4 (4 heads each).
No collectives: every core computes its own [s, 256] output slice.

Host-side marshalling:
  - weight columns permuted so each head's 64 columns are contiguous
  - x[b] pre-transposed to xT [D, S] (PE contracts over partitions, so x
    must be d-major; transposing on host is free)
  - bf16 casts for all matmul operands (fp32 PE matmul is multi-pass slow)

Device-side (per core, SPMD):
  - QT[dq, s] = Wq.T @ x.T (+bias), KT likewise, V[s, dv] with an extra
    ones column per head (V_aug) so PV also produces softmax row sums
  - per head: ST[j, i] = KT_h.T @ QT_h (K=64); E = exp(ST/32) on ScalarE
    straight out of PSUM (scores are tiny, |s| < ~0.3, so softmax without
    max subtraction is numerically safe)
  - OT_aug[65, i] += V_aug[j,:].T @ E[j, i] accumulated over j tiles;
    row 64 = sum_j E (softmax denominator)
  - normalize: DVE reciprocal of row 64, GPSIMD partition_broadcast,
    DVE multiply; DMA out OT [64(c), 2048(s)] per head (host transposes)
"""

import sys
import os

for _p in ("/opt/trn_rl_repo",):
    if os.path.isdir(_p) and _p not in sys.path:
        sys.path.insert(0, _p)

import numpy as np
import ml_dtypes
from contextlib import ExitStack

import concourse.bass as bass
import concourse.mybir as mybir
import concourse.tile as tile
from concourse import bacc
from concourse.bass_utils import run_bass_kernel_spmd

BF16 = mybir.dt.bfloat16
F32 = mybir.dt.float32
NPBF16 = ml_dtypes.bfloat16

B, S, D, H = 2, 2048, 1024, 16
NCORES = 8
HGROUPS = 4              # tensor-parallel ways over heads
NH_LOC = H // HGROUPS    # 4 heads per core
DH = D // H              # 64
DQ = NH_LOC * DH         # 256 projection cols per core
KT = D // 128            # 8 contraction tiles
SCALE = 1.0 / 32.0       # 1/sqrt(D)

# column permutation: permuted col h*64+c  <-  original col c*16+h
PERM = np.array([c * H + h for h in range(H) for c in range(DH)], dtype=np.int64)


def build_bass():
    nc = bacc.Bacc("TRN2", target_bir_lowering=False)
    xT_d = nc.dram_tensor("xT", [D, S], BF16, kind="ExternalInput")
    wq_d = nc.dram_tensor("wq", [D, DQ], BF16, kind="ExternalInput")
    wk_d = nc.dram_tensor("wk", [D, DQ], BF16, kind="ExternalInput")
    wv_d = nc.dram_tensor("wv", [D, DQ], BF16, kind="ExternalInput")
    bqT_d = nc.dram_tensor("bqT", [DQ, 1], F32, kind="ExternalInput")
    out_d = nc.dram_tensor("out", [DQ, S], F32, kind="ExternalOutput")

    with ExitStack() as ctx:
        tc = ctx.enter_context(tile.TileContext(nc))
        consts = ctx.enter_context(tc.tile_pool(name="consts", bufs=1))
        xpool = ctx.enter_context(tc.tile_pool(name="xpool", bufs=KT))
        epool = ctx.enter_context(tc.tile_pool(name="epool", bufs=8))
        npool = ctx.enter_context(tc.tile_pool(name="npool", bufs=2))
        opool = ctx.enter_context(tc.tile_pool(name="opool", bufs=2))
        # one flat PSUM layout, no nested pools (nested release would
        # serialize attention behind all projection work):
        # pmain: 2 bufs x [128,1024] (2 banks each) shared by projection
        # accumulators and score tiles; pov: [65,2048] PV accumulator (4 banks)
        pmain = ctx.enter_context(tc.tile_pool(name="pmain", bufs=2, space="PSUM"))
        pov = ctx.enter_context(tc.tile_pool(name="pov", bufs=1, space="PSUM"))

        # ---- load inputs ----
        xT_sb = [xpool.tile([128, S], BF16, tag="xT", name=f"xT{_i}") for _i in range(KT)]
        for kt in range(KT):
            for hh in range(2):
                nc.sync.dma_start(
                    out=xT_sb[kt][:, hh * (S // 2):(hh + 1) * (S // 2)],
                    in_=xT_d[kt * 128:(kt + 1) * 128, hh * (S // 2):(hh + 1) * (S // 2)])

        wq_sb = consts.tile([128, KT, DQ], BF16)
        wk_sb = consts.tile([128, KT, DQ], BF16)
        wv_sb = consts.tile([128, KT, DQ], BF16)
        nc.sync.dma_start(out=wq_sb[:], in_=wq_d.ap().rearrange("(t p) n -> p t n", p=128))
        nc.sync.dma_start(out=wk_sb[:], in_=wk_d.ap().rearrange("(t p) n -> p t n", p=128))
        nc.sync.dma_start(out=wv_sb[:], in_=wv_d.ap().rearrange("(t p) n -> p t n", p=128))
        bq_sb = consts.tile([128, 2, 1], F32)
        nc.sync.dma_start(out=bq_sb[:], in_=bqT_d.ap().rearrange("(t p) o -> p t o", p=128))

        qt_sb = consts.tile([128, 2, S], BF16)
        kt_sb = consts.tile([128, 2, S], BF16)
        v_sb = consts.tile([128, 16, NH_LOC * (DH + 1)], BF16)
        nc.vector.memset(v_sb[:], 1.0)

        def proj_q(m, ic):
            ps = pmain.tile([128, 512], F32, tag="pm", name="psq")
            for kt in range(KT):
                nc.tensor.matmul(
                    ps[:],
                    lhsT=wq_sb[:, kt, m * 128:(m + 1) * 128],
                    rhs=xT_sb[kt][:, ic * 512:(ic + 1) * 512],
                    start=(kt == 0), stop=(kt == KT - 1),
                )
            nc.vector.tensor_scalar_add(
                qt_sb[:, m, ic * 512:(ic + 1) * 512], ps[:], bq_sb[:, m, :])

        def proj_k(m, ic):
            ps = pmain.tile([128, 512], F32, tag="pm", name="psk")
            for kt in range(KT):
                nc.tensor.matmul(
                    ps[:],
                    lhsT=wk_sb[:, kt, m * 128:(m + 1) * 128],
                    rhs=xT_sb[kt][:, ic * 512:(ic + 1) * 512],
                    start=(kt == 0), stop=(kt == KT - 1),
                )
            nc.vector.tensor_copy(out=kt_sb[:, m, ic * 512:(ic + 1) * 512], in_=ps[:])

        def proj_v(st):
            """project V for s-tile st into v_sb (leaving the ones columns)"""
            ps = pmain.tile([128, 512], F32, tag="pm", name="psv")
            for kt in range(KT):
                nc.tensor.matmul(
                    ps[:, 0:DQ],
                    lhsT=xT_sb[kt][:, st * 128:(st + 1) * 128],
                    rhs=wv_sb[:, kt, :],
                    start=(kt == 0), stop=(kt == KT - 1),
                )
            nc.vector.tensor_copy(
                out=v_sb[:, st, :].rearrange("p (h e) -> p h e", e=DH + 1)[:, :, 0:DH],
                in_=ps[:, 0:DQ].rearrange("p (h c) -> p h c", c=DH),
            )

        # QT/KT tile m=0 up front (heads 0/1 depend on it) ...
        for ic in range(4):
            proj_q(0, ic)
            proj_k(0, ic)
        m1_chunks = [f(1, ic) for ic in range(4) for f in (lambda m, i: (proj_q, i), lambda m, i: (proj_k, i))]

        # ... then heads, with the remaining projection work interleaved into
        # the first two head loops so PE stays dense while ACT runs exp.
        for h in range(NH_LOC):
            m = h // 2
            off = (h % 2) * DH
            o_ps = pov.tile([DH + 1, S], F32, tag="ov", name="ops")
            for jc in range(16):
                if h == 0:
                    proj_v(jc)                 # PV(h=0, jc) needs exactly this
                elif h == 1 and jc % 2 == 0:
                    fn, ic = m1_chunks[jc // 2]
                    fn(1, ic)                  # heads 2/3 inputs, one chunk/2jc
                for half in range(2):
                    s_ps = pmain.tile([128, 1024], F32, tag="pm", name="pss")
                    for ic2 in range(2):
                        i0 = half * 1024 + ic2 * 512
                        nc.tensor.matmul(
                            s_ps[:, ic2 * 512:(ic2 + 1) * 512],
                            lhsT=kt_sb[off:off + DH, m, jc * 128:(jc + 1) * 128],
                            rhs=qt_sb[off:off + DH, m, i0:i0 + 512],
                            start=True, stop=True,
                        )
                    e_sb = epool.tile([128, 1024], BF16, tag="e", name="esb")
                    nc.scalar.activation(
                        e_sb[:], s_ps[:], mybir.ActivationFunctionType.Exp, scale=SCALE)
                    for ic2 in range(2):
                        i0 = half * 1024 + ic2 * 512
                        nc.tensor.matmul(
                            o_ps[:, i0:i0 + 512],
                            lhsT=v_sb[:, jc, h * (DH + 1):(h + 1) * (DH + 1)],
                            rhs=e_sb[:, ic2 * 512:(ic2 + 1) * 512],
                            start=(jc == 0), stop=(jc == 15),
                        )
            # single copy releases the PV accumulator banks fast; the rest
            # of the normalization runs from SBUF off the PE/ACT critical path
            o_sb = opool.tile([DH + 1, S], F32, tag="osb")
            nc.vector.tensor_copy(out=o_sb[:], in_=o_ps[:])
            rl_sb = npool.tile([1, S], F32, tag="rl")
            nc.vector.reciprocal(rl_sb[:], o_sb[DH:DH + 1, :])
            rb_sb = npool.tile([DH, S], F32, tag="rb")
            nc.gpsimd.partition_broadcast(rb_sb[:], rl_sb[:])
            ost = opool.tile([DH, S], F32, tag="ost")
            nc.vector.tensor_mul(ost[:], o_sb[0:DH, :], rb_sb[:])
            nc.sync.dma_start(out=out_d[h * DH:(h + 1) * DH, :], in_=ost[:])

    nc.finalize()
    return nc


_NC_CACHE = None


def _get_nc():
    global _NC_CACHE
    if _NC_CACHE is None:
        _NC_CACHE = build_bass()
    return _NC_CACHE


def kernel(x, Wq, Bq, Wk, Wv, n_heads=16, **_ignored):
    x = np.asarray(x, dtype=np.float32)
    Wq = np.asarray(Wq, dtype=np.float32)
    Bq = np.asarray(Bq, dtype=np.float32).reshape(-1)
    Wk = np.asarray(Wk, dtype=np.float32)
    Wv = np.asarray(Wv, dtype=np.float32)

    wq_p = Wq[:, PERM]
    wk_p = Wk[:, PERM]
    wv_p = Wv[:, PERM]
    bq_p = Bq[PERM]

    xT = [np.ascontiguousarray(x[b].T).astype(NPBF16) for b in range(B)]
    in_maps = []
    for core in range(NCORES):
        b, hg = core // HGROUPS, core % HGROUPS
        sl = slice(hg * DQ, (hg + 1) * DQ)
        in_maps.append({
            "xT": xT[b],
            "wq": np.ascontiguousarray(wq_p[:, sl]).astype(NPBF16),
            "wk": np.ascontiguousarray(wk_p[:, sl]).astype(NPBF16),
            "wv": np.ascontiguousarray(wv_p[:, sl]).astype(NPBF16),
            "bqT": np.ascontiguousarray(bq_p[sl]).reshape(DQ, 1).astype(np.float32),
        })

    nc = _get_nc()
    res = run_bass_kernel_spmd(nc, in_maps, core_ids=list(range(NCORES)))

    out = np.empty((B, S, D), dtype=np.float32)
    for b in range(B):
        big = np.concatenate(
            [res.results[b * HGROUPS + hg]["out"] for hg in range(HGROUPS)], axis=0)
        out[b][:, PERM] = big.T
    return out


# revision 7
# speedup vs baseline: 1.8734x; 1.0990x over previous
"""Distributed multi-head attention kernel for one TRN2 chip (8 NeuronCores).

Problem: B=2, S=2048, D=1024, H=16 heads (dh=64), interleaved head split
(reshape d -> (dh, H) with heads LAST), scale = 1/sqrt(D).

Sharding: core c => batch b = c//4, head-group hg = c%4 (4 heads each).
No collectives: every core computes its own [s, 256] output slice.

Host-side marshalling:
  - weight columns permuted so each head's 64 columns are contiguous
  - x[b] pre-transposed to xT [D, S] (PE contracts over partitions, so x
    must be d-major; transposing on host is free)
  - bf16 casts for all matmul operands (fp32 PE matmul is multi-pass slow)

Device-side (per core, SPMD):
  - QT[dq, s] = Wq.T @ x.T (+bias), KT likewise, V[s, dv] with an extra
    ones column per head (V_aug) so PV also produces softmax row sums
  - per head: ST[j, i] = KT_h.T @ QT_h (K=64); E = exp(ST/32) on ScalarE
    straight out of PSUM (scores are tiny, |s| < ~0.3, so softmax without
    max subtraction is numerically safe)
  - OT_aug[65, i] += V_aug[j,:].T @ E[j, i] accumulated over j tiles;
    row 64 = sum_j E (softmax denominator)
  - normalize: DVE reciprocal of row 64, GPSIMD partition_broadcast,
    DVE multiply; DMA out OT [64(c), 2048(s)] per head (host transposes)
"""

import sys
import os

for _p in ("/opt/trn_rl_repo",):
    if os.path.isdir(_p) and _p not in sys.path:
        sys.path.insert(0, _p)

import numpy as np
import ml_dtypes
from contextlib import ExitStack

import concourse.bass as bass
import concourse.mybir as mybir
import concourse.tile as tile
from concourse import bacc
from concourse.bass_utils import run_bass_kernel_spmd

BF16 = mybir.dt.bfloat16
F32 = mybir.dt.float32
NPBF16 = ml_dtypes.bfloat16

B, S, D, H = 2, 2048, 1024, 16
NCORES = 8
HGROUPS = 4              # tensor-parallel ways over heads
NH_LOC = H // HGROUPS    # 4 heads per core
DH = D // H              # 64
DQ = NH_LOC * DH         # 256 projection cols per core
KT = D // 128            # 8 contraction tiles
SCALE = 1.0 / 32.0       # 1/sqrt(D)

# column permutation: permuted col h*64+c  <-  original col c*16+h
PERM = np.array([c * H + h for h in range(H) for c in range(DH)], dtype=np.int64)


def build_bass():
    nc = bacc.Bacc("TRN2", target_bir_lowering=False)
    xT_d = nc.dram_tensor("xT", [D, S], BF16, kind="ExternalInput")
    wq_d = nc.dram_tensor("wq", [D, DQ], BF16, kind="ExternalInput")
    wk_d = nc.dram_tensor("wk", [D, DQ], BF16, kind="ExternalInput")
    wv_d = nc.dram_tensor("wv", [D, DQ], BF16, kind="ExternalInput")
    bqT_d = nc.dram_tensor("bqT", [DQ, 1], F32, kind="ExternalInput")
    out_d = nc.dram_tensor("out", [DQ, S], F32, kind="ExternalOutput")

    with ExitStack() as ctx:
        tc = ctx.enter_context(tile.TileContext(nc))
        consts = ctx.enter_context(tc.tile_pool(name="consts", bufs=1))
        xpool = ctx.enter_context(tc.tile_pool(name="xpool", bufs=KT))
        epool = ctx.enter_context(tc.tile_pool(name="epool", bufs=8))
        npool = ctx.enter_context(tc.tile_pool(name="npool", bufs=2))
        opool = ctx.enter_context(tc.tile_pool(name="opool", bufs=2))
        # one flat PSUM layout, no nested pools (nested release would
        # serialize attention behind all projection work):
        # pmain: 2 bufs x [128,1024] (2 banks each) shared by projection
        # accumulators and score tiles; pov: [65,2048] PV accumulator (4 banks)
        pmain = ctx.enter_context(tc.tile_pool(name="pmain", bufs=2, space="PSUM"))
        pov = ctx.enter_context(tc.tile_pool(name="pov", bufs=1, space="PSUM"))

        # ---- load inputs ----
        xT_sb = [xpool.tile([128, S], BF16, tag="xT", name=f"xT{_i}") for _i in range(KT)]
        wq_sb = consts.tile([128, KT, DQ], BF16)
        wk_sb = consts.tile([128, KT, DQ], BF16)
        wv_sb = consts.tile([128, KT, DQ], BF16)
        bq_sb = consts.tile([128, 2, 1], F32)
        # order: what unblocks the first exp soonest comes first
        nc.sync.dma_start(out=wq_sb[:], in_=wq_d.ap().rearrange("(t p) n -> p t n", p=128))
        nc.sync.dma_start(out=wk_sb[:], in_=wk_d.ap().rearrange("(t p) n -> p t n", p=128))
        nc.sync.dma_start(out=bq_sb[:], in_=bqT_d.ap().rearrange("(t p) o -> p t o", p=128))
        for kt in range(KT):
            nc.sync.dma_start(out=xT_sb[kt][:, 0:S // 2],
                              in_=xT_d[kt * 128:(kt + 1) * 128, 0:S // 2])
        nc.sync.dma_start(out=wv_sb[:], in_=wv_d.ap().rearrange("(t p) n -> p t n", p=128))
        for kt in range(KT):
            nc.sync.dma_start(out=xT_sb[kt][:, S // 2:S],
                              in_=xT_d[kt * 128:(kt + 1) * 128, S // 2:S])

        qt_sb = consts.tile([128, 2, S], BF16)
        kt_sb = consts.tile([128, 2, S], BF16)
        v_sb = consts.tile([128, 16, NH_LOC * (DH + 1)], BF16)
        nc.vector.memset(v_sb[:], 1.0)

        def proj_q(m, ic):
            ps = pmain.tile([128, 512], F32, tag="pm", name="psq")
            for kt in range(KT):
                nc.tensor.matmul(
                    ps[:],
                    lhsT=wq_sb[:, kt, m * 128:(m + 1) * 128],
                    rhs=xT_sb[kt][:, ic * 512:(ic + 1) * 512],
                    start=(kt == 0), stop=(kt == KT - 1),
                )
            nc.vector.tensor_scalar_add(
                qt_sb[:, m, ic * 512:(ic + 1) * 512], ps[:], bq_sb[:, m, :])

        def proj_k(m, ic):
            ps = pmain.tile([128, 512], F32, tag="pm", name="psk")
            for kt in range(KT):
                nc.tensor.matmul(
                    ps[:],
                    lhsT=wk_sb[:, kt, m * 128:(m + 1) * 128],
                    rhs=xT_sb[kt][:, ic * 512:(ic + 1) * 512],
                    start=(kt == 0), stop=(kt == KT - 1),
                )
            nc.vector.tensor_copy(out=kt_sb[:, m, ic * 512:(ic + 1) * 512], in_=ps[:])

        def proj_v(st):
            """project V for s-tile st into v_sb (leaving the ones columns)"""
            ps = pmain.tile([128, 512], F32, tag="pm", name="psv")
            for kt in range(KT):
                nc.tensor.matmul(
                    ps[:, 0:DQ],
                    lhsT=xT_sb[kt][:, st * 128:(st + 1) * 128],
                    rhs=wv_sb[:, kt, :],
                    start=(kt == 0), stop=(kt == KT - 1),
                )
            nc.vector.tensor_copy(
                out=v_sb[:, st, :].rearrange("p (h e) -> p h e", e=DH + 1)[:, :, 0:DH],
                in_=ps[:, 0:DQ].rearrange("p (h c) -> p h c", c=DH),
            )

        # QT/KT tile m=0 up front (heads 0/1 depend on it) ...
        for ic in range(4):
            proj_q(0, ic)
            proj_k(0, ic)
        # (q/k alternation keeps first-score deps early)
        m1_chunks = [f(1, ic) for ic in range(4) for f in (lambda m, i: (proj_q, i), lambda m, i: (proj_k, i))]

        # ... then heads, with the remaining projection work interleaved into
        # the first two head loops so PE stays dense while ACT runs exp.
        for h in range(NH_LOC):
            m = h // 2
            off = (h % 2) * DH
            o_ps = pov.tile([DH + 1, S], F32, tag="ov", name="ops")
            for jc in range(16):
                if h == 0:
                    proj_v(jc)                 # PV(h=0, jc) needs exactly this
                elif h == 1 and jc % 2 == 0:
                    fn, ic = m1_chunks[jc // 2]
                    fn(1, ic)                  # heads 2/3 inputs, one chunk/2jc
                for half in range(2):
                    s_ps = pmain.tile([128, 1024], F32, tag="pm", name="pss")
                    for ic2 in range(2):
                        i0 = half * 1024 + ic2 * 512
                        nc.tensor.matmul(
                            s_ps[:, ic2 * 512:(ic2 + 1) * 512],
                            lhsT=kt_sb[off:off + DH, m, jc * 128:(jc + 1) * 128],
                            rhs=qt_sb[off:off + DH, m, i0:i0 + 512],
                            start=True, stop=True,
                        )
                    e_sb = epool.tile([128, 1024], BF16, tag="e", name="esb")
                    nc.scalar.activation(
                        e_sb[:], s_ps[:], mybir.ActivationFunctionType.Exp, scale=SCALE)
                    for ic2 in range(2):
                        i0 = half * 1024 + ic2 * 512
                        nc.tensor.matmul(
                            o_ps[:, i0:i0 + 512],
                            lhsT=v_sb[:, jc, h * (DH + 1):(h + 1) * (DH + 1)],
                            rhs=e_sb[:, ic2 * 512:(ic2 + 1) * 512],
                            start=(jc == 0), stop=(jc == 15),
                        )
            # single copy releases the PV accumulator banks fast; the rest
            # of the normalization runs from SBUF off the PE/ACT critical
            # path, chunked in halves so the tail latency stays short
            o_sb = opool.tile([DH, S], F32, tag="osb")
            rl_sb = npool.tile([1, S], F32, tag="rl")
            rb_sb = npool.tile([DH, S], F32, tag="rb")
            ost = opool.tile([DH, S], F32, tag="ost")
            rl2_sb = npool.tile([1, S], F32, tag="rl2")
            for hh in range(2):
                sl = slice(hh * (S // 2), (hh + 1) * (S // 2))
                nc.vector.tensor_copy(out=rl_sb[0:1, sl], in_=o_ps[DH:DH + 1, sl])
                nc.vector.tensor_copy(out=o_sb[0:DH, sl], in_=o_ps[0:DH, sl])
                nc.vector.reciprocal_approx_fast(
                    out=rl2_sb[0:1, sl], in_=rl_sb[0:1, sl])
                nc.gpsimd.partition_broadcast(rb_sb[:, sl], rl2_sb[0:1, sl])
                nc.vector.tensor_mul(ost[:, sl], o_sb[0:DH, sl], rb_sb[:, sl])
                nc.sync.dma_start(out=out_d[h * DH:(h + 1) * DH, sl], in_=ost[:, sl])

    nc.finalize()
    return nc


_NC_CACHE = None


def _get_nc():
    global _NC_CACHE
    if _NC_CACHE is None:
        _NC_CACHE = build_bass()
    return _NC_CACHE


def kernel(x, Wq, Bq, Wk, Wv, n_heads=16, **_ignored):
    x = np.asarray(x, dtype=np.float32)
    Wq = np.asarray(Wq, dtype=np.float32)
    Bq = np.asarray(Bq, dtype=np.float32).reshape(-1)
    Wk = np.asarray(Wk, dtype=np.float32)
    Wv = np.asarray(Wv, dtype=np.float32)

    wq_p = Wq[:, PERM]
    wk_p = Wk[:, PERM]
    wv_p = Wv[:, PERM]
    bq_p = Bq[PERM]

    xT = [np.ascontiguousarray(x[b].T).astype(NPBF16) for b in range(B)]
    in_maps = []
    for core in range(NCORES):
        b, hg = core // HGROUPS, core % HGROUPS
        sl = slice(hg * DQ, (hg + 1) * DQ)
        in_maps.append({
            "xT": xT[b],
            "wq": np.ascontiguousarray(wq_p[:, sl]).astype(NPBF16),
            "wk": np.ascontiguousarray(wk_p[:, sl]).astype(NPBF16),
            "wv": np.ascontiguousarray(wv_p[:, sl]).astype(NPBF16),
            "bqT": np.ascontiguousarray(bq_p[sl]).reshape(DQ, 1).astype(np.float32),
        })

    nc = _get_nc()
    res = run_bass_kernel_spmd(nc, in_maps, core_ids=list(range(NCORES)))

    out = np.empty((B, S, D), dtype=np.float32)
    for b in range(B):
        big = np.concatenate(
            [res.results[b * HGROUPS + hg]["out"] for hg in range(HGROUPS)], axis=0)
        out[b][:, PERM] = big.T
    return out
